# revision 68
# baseline (speedup 1.0000x reference)
"""Trainium2 Bass kernel for nn_AttnGCNNet (GCN message passing + attention + conv).

Strategy (8 NeuronCores, SPMD single NEFF):
  - Nodes/graphs sharded by contiguous ranges: core c owns nodes [c*4096, (c+1)*4096)
    = 16 whole graphs. Edges sharded by dst node.
  - GCN layer: agg = A_norm @ h done as: dma_gather of h rows (by edge src, dst-sorted)
    from a replicated HBM copy of h, then PE one-hot scatter matmuls into PSUM
    (transposed layout [d, nodes]); self-loop term via block-diag one-hot from an
    SBUF node-major copy; then (agg)@W + b + relu on PE/ACT; node-major write-back via
    PE transpose; AllGather replicates the new h shard to all cores' HBM.
  - Attention/conv/MLP head: per-graph matmuls in transposed layouts; softmax along
    free axis with PE-computed row sums; conv as 64 accumulating matmuls.
  - Host<->device transfer minimized (the axon tunnel is ~60 MB/s): x is sent as
    bf16 [4096,78] per-core shards, converted to f32 and AllGathered on device;
    the edge/self/target one-hots (eoh/soh/toh) and the identity are built on
    device from compact index/weight arrays (iota + fused compare*weight); the
    replicated weights travel as 1/8 shards of two packed blobs (f32 + bf16,
    head MLP weights in bf16) and are AllGathered on device.
  - Per-instruction overhead in this environment is ~10us, so instruction count
    is minimized: GCN post-ops batched 4 groups per PSUM tile, conv batched 4
    graphs per accumulation chain, blob loads one DMA per tensor.
  - kernel() enables the jax persistent compilation cache: run_bass_via_pjrt
    re-jits a fresh closure per call, and without the cache every call repays
    ~0.9s of BIR verify + NEFF wrap even with a warm neuronxcc cache.
"""
import os
import time
import numpy as np

B, N_PER, E, SEQ, VOCAB, D = 128, 256, 262144, 1000, 26, 128
N = B * N_PER
GCN_DIMS = [(78, 78), (78, 88), (88, 88), (88, 118), (118, 118), (118, 128)]
NCORES = 8
NSHARD = N // NCORES       # 4096
GROUPS = NSHARD // 128     # 32
GPC = B // NCORES          # 16 graphs per core
CHUNK_TILES = 40           # edge tiles per dma_gather call


# ---------------------------------------------------------------- host packing
def _pack_edges(src, dst, enorm):
    """Shared-structure edge tiling. Returns (structure, per-core arrays)."""
    per_core = []
    for c in range(NCORES):
        lo = c * NSHARD
        sel = np.nonzero((dst >= lo) & (dst < lo + NSHARD))[0]
        ed = (dst[sel] - lo).astype(np.int64)
        order = np.argsort(ed, kind="stable")
        per_core.append((src[sel][order], ed[order], enorm[sel][order]))

    # group tile counts (shared across cores)
    NT = []
    cge = [[None] * GROUPS for _ in range(NCORES)]
    for g in range(GROUPS):
        cnt = 1
        for c in range(NCORES):
            es, ed, ew = per_core[c]
            m = (ed >= g * 128) & (ed < (g + 1) * 128)
            cge[c][g] = (es[m], ed[m] - g * 128, ew[m])
            cnt = max(cnt, (len(ew[m]) + 127) // 128)
        NT.append(cnt)

    # union windows per (g, t)
    tiles = []  # flat list of (g, t, off, wid, col)
    col = 0
    for g in range(GROUPS):
        for t in range(NT[g]):
            lo_w, hi_w = 128, -1
            for c in range(NCORES):
                ged = cge[c][g][1][t * 128:(t + 1) * 128]
                if len(ged):
                    lo_w = min(lo_w, int(ged.min()))
                    hi_w = max(hi_w, int(ged.max()))
            if hi_w < 0:
                lo_w, hi_w = 0, 0
            wid = hi_w - lo_w + 1
            tiles.append((g, t, lo_w, wid, col))
            col += wid
    sumw = col
    T_tot = len(tiles)

    gidx, ecols, ews = [], [], []
    for c in range(NCORES):
        all_idx = np.zeros(T_tot * 128, dtype=np.int16)
        ecol = np.zeros((128, T_tot), dtype=np.uint8)
        ew = np.zeros((128, T_tot), dtype=np.float32)
        for fi, (g, t, off, wid, colo) in enumerate(tiles):
            ges, ged, gew = cge[c][g]
            sl = slice(t * 128, (t + 1) * 128)
            e_s, e_d, e_w = ges[sl], ged[sl], gew[sl]
            n = len(e_s)
            all_idx[fi * 128:fi * 128 + n] = e_s.astype(np.int16)
            if n:
                ecol[:n, fi] = (e_d - off).astype(np.uint8)
                ew[:n, fi] = e_w
        C = T_tot * 8
        wrapped = all_idx.reshape(C, 16).T  # [16, C]
        gidx.append(np.ascontiguousarray(wrapped))
        ecols.append(ecol)
        ews.append(ew)
    return dict(NT=NT, tiles=tiles, sumw=sumw, T_tot=T_tot), gidx, ecols, ews


def _blob_pack(tensors):
    """Pack [P<=128, C] matrices into a stack of [P,128] blocks.

    Returns (blob [R,128], layout name -> (row0, P, C))."""
    layout = {}
    blocks = []
    row = 0
    for name, a in tensors:
        P, Cc = a.shape
        nb = (Cc + 127) // 128
        for k in range(nb):
            blk = np.zeros((P, 128), a.dtype)
            cw = min(128, Cc - 128 * k)
            blk[:, :cw] = a[:, 128 * k:128 * k + cw]
            blocks.append(blk)
        layout[name] = (row, P, Cc)
        row += nb * P
    if row % NCORES:
        blocks.append(np.zeros((NCORES - row % NCORES, 128), tensors[0][1].dtype))
    blob = np.ascontiguousarray(np.concatenate(blocks, axis=0))
    assert blob.shape[0] % NCORES == 0, blob.shape
    return blob, layout


def _host_prep(inputs):
    import ml_dtypes
    bf16 = np.dtype(ml_dtypes.bfloat16)
    inp = {k: np.asarray(v) for k, v in inputs.items()}
    x = inp["x"].astype(np.float32)
    src = inp["edge_index"][0].astype(np.int64)
    dst = inp["edge_index"][1].astype(np.int64)
    target = inp["target"].astype(np.int64)

    deg = 1.0 + np.bincount(dst, minlength=N).astype(np.float32)
    dis = (1.0 / np.sqrt(deg)).astype(np.float32)
    enorm = (dis[src] * dis[dst]).astype(np.float32)
    snorm = (dis * dis).astype(np.float32)

    struct, gidx, ecols, ews = _pack_edges(src, dst, enorm)

    # per-core snorm shard in [128 part, 32 group] layout
    snorms = []
    for c in range(NCORES):
        s = snorm[c * NSHARD:(c + 1) * NSHARD].reshape(GROUPS, 128).T
        snorms.append(np.ascontiguousarray(s.astype(np.float32)))

    # GCN weights [128, 6*128], biases [128, 6]
    Wt = np.zeros((128, 6 * 128), dtype=np.float32)
    bias = np.zeros((128, 6), dtype=np.float32)
    for l, (di, do) in enumerate(GCN_DIMS, 0):
        Wt[:di, l * 128:l * 128 + do] = inp["W%d" % (l + 1)].astype(np.float32)
        bias[:do, l] = inp["b%d" % (l + 1)].astype(np.float32)

    qemb = (inp["emb"].astype(np.float32) @ inp["attn_in_w"].astype(np.float32))
    qemb_bf = np.zeros((32, 128), dtype=bf16)
    qemb_bf[:26, :] = qemb.astype(bf16)
    W_updown = np.zeros((128, 256), dtype=bf16)
    W_updown[:, 0:128] = inp["attn_out_w"][:128, :].astype(bf16)
    W_updown[:, 128:256] = inp["attn_out_w"][128:, :].astype(bf16)

    # target indices per core [16, 1000] (one-hots are built on device)
    tgts = []
    for c in range(NCORES):
        t = target[c * GPC:(c + 1) * GPC].astype(np.uint8)
        tgts.append(np.ascontiguousarray(t))

    sumsel = np.zeros((128, 16 * GPC), dtype=bf16)
    for b in range(GPC):
        sumsel[:, 16 * b + b] = 1.0
    bsel = np.zeros((16, 128 * GPC), dtype=np.float32)
    for b in range(GPC):
        bsel[b, 128 * b:128 * (b + 1)] = 1.0

    # conv weights [128, 8*8*32] bf16 (k-major, then ic-tile), zero-padded ic
    cw = inp["conv_w"].astype(np.float32)  # [32, 1000, 8]
    convw = np.zeros((128, 8 * 8 * 32), dtype=np.float32)
    for k in range(8):
        for it in range(8):
            icn = min(128, 1000 - it * 128)
            blk = cw[:, it * 128:it * 128 + icn, k]  # [32, icn]
            convw[:icn, (k * 8 + it) * 32:(k * 8 + it) * 32 + 32] = blk.T
    convw = convw.astype(bf16)
    convb = np.zeros((32, 1), dtype=np.float32)
    convb[:, 0] = inp["conv_b"].astype(np.float32)

    f = lambda a: a.astype(np.float32)
    fc1xtw = np.zeros((128, 32 * 128), dtype=np.float32)
    for oc in range(32):
        fc1xtw[:121, oc * 128:(oc + 1) * 128] = f(inp["fc1xt_w"])[oc * 121:(oc + 1) * 121, :]
    fc1xtb = f(inp["fc1xt_b"]).reshape(128, 1)

    fcg1w = f(inp["fcg1_w"])                    # [128, 1024]
    fcg1b = np.stack([f(inp["fcg1_b"])[128 * j:128 * (j + 1)] for j in range(8)], axis=1)
    fcg2w = np.concatenate([f(inp["fcg2_w"])[128 * j:128 * (j + 1), :] for j in range(8)], axis=1)  # [128, 1024]
    fcg2b = f(inp["fcg2_b"]).reshape(128, 1)
    fc1w = np.concatenate([f(inp["fc1_w"])[128 * k:128 * (k + 1), :] for k in range(2)], axis=1)  # [128, 2048]
    fc1b = np.stack([f(inp["fc1_b"])[128 * j:128 * (j + 1)] for j in range(8)], axis=1)  # [128, 8]
    fc2w = np.concatenate([f(inp["fc2_w"])[128 * k:128 * (k + 1), :] for k in range(8)], axis=1)  # [128, 4096]
    fc2b = np.stack([f(inp["fc2_b"])[128 * j:128 * (j + 1)] for j in range(4)], axis=1)  # [128, 4]
    outw = np.concatenate([f(inp["out_w"])[128 * k:128 * (k + 1), :] for k in range(4)], axis=1)  # [128, 4]
    outb = float(np.asarray(inp["out_b"]).reshape(-1)[0])

    wblob, wlay = _blob_pack([
        ("Wt", Wt), ("bias", bias), ("fc1xtb", fc1xtb),
        ("fcg1b", fcg1b), ("fcg2b", fcg2b),
        ("fc1b", fc1b), ("fc2b", fc2b),
        ("outw", outw), ("convb", convb), ("bsel", bsel),
    ])
    bblob, blay = _blob_pack([
        ("qemb", qemb_bf), ("W_updown", W_updown), ("sumsel", sumsel),
        ("convw", convw),
        ("fc1xtw", fc1xtw.astype(bf16)),
        ("fcg1w", fcg1w.astype(bf16)), ("fcg2w", fcg2w.astype(bf16)),
        ("fc1w", fc1w.astype(bf16)), ("fc2w", fc2w.astype(bf16)),
    ])
    struct["wlay"], struct["wrows"] = wlay, wblob.shape[0]
    struct["blay"], struct["brows"] = blay, bblob.shape[0]

    # consolidate all per-core bf16/raw data into one input blob (fewer
    # host->device arrays: per-array transfer overhead is real on this link);
    # x goes separately as int8 (absmax-scaled) to halve its bytes
    xscale = float(np.abs(x).max()) / 127.0
    xq = np.clip(np.round(x / xscale), -127, 127).astype(np.int8)
    per_core = []
    rf, rb = wblob.shape[0] // NCORES, bblob.shape[0] // NCORES
    doff = {}
    for c in range(NCORES):
        parts = [
            ("ew", ews[c].astype(bf16)),
            ("ecol", ecols[c].astype(bf16)),
            ("tgt", tgts[c].astype(bf16)),
            ("gidx", gidx[c].view(bf16)),
            ("snorm", snorms[c].astype(bf16)),
            ("bshard", bblob[c * rb:(c + 1) * rb]),
        ]
        cur, flats = 0, []
        for name, a in parts:
            fl = np.ascontiguousarray(a).reshape(-1)
            assert fl.size % 128 == 0, (name, fl.size)
            doff[name] = cur
            cur += fl.size
            flats.append(fl)
        dblob = np.concatenate(flats).reshape(-1, 128)
        per_core.append(dict(
            dblob=dblob,
            xq=np.ascontiguousarray(xq[c * NSHARD:(c + 1) * NSHARD, :]),
            wshard=np.ascontiguousarray(wblob[c * rf:(c + 1) * rf])))
    struct["doff"], struct["drows"] = doff, per_core[0]["dblob"].shape[0]
    struct["xscale"] = xscale
    return struct, per_core, outb


# ---------------------------------------------------------------- device build
def _build(struct, outb):
    import concourse.bacc as bacc
    import concourse.bass as bass
    import concourse.tile as tile
    import concourse.mybir as mybir

    f32 = mybir.dt.float32
    f32r = mybir.dt.float32r
    bf = mybir.dt.bfloat16
    i16 = mybir.dt.int16
    u8 = mybir.dt.uint8
    AF = mybir.ActivationFunctionType
    ALU = mybir.AluOpType
    tiles, T_tot, sumw = struct["tiles"], struct["T_tot"], struct["sumw"]
    wlay, blay = struct["wlay"], struct["blay"]
    doff = struct["doff"]

    nc = bacc.Bacc("TRN2", target_bir_lowering=False, debug=False, num_devices=NCORES)

    # I/O: one bf16 blob with all per-core data + int8 x + the f32 weight shard
    i8 = mybir.dt.int8
    ein = lambda n, s, d: nc.dram_tensor(n, s, d, kind="ExternalInput")
    dblob_d = ein("dblob", [struct["drows"], 128], bf)
    xq_d = ein("xq", [NSHARD, 78], i8)
    wshard_d = ein("wshard", [struct["wrows"] // NCORES, 128], f32)
    dflat = dblob_d.ap().rearrange("r c -> (r c)")
    x_nm_ap = xq_d.ap().rearrange("(g p) c -> p g c", p=128)
    ew_ap = dflat[doff["ew"]:doff["ew"] + 128 * T_tot].rearrange(
        "(p t) -> p t", p=128)
    ecol_ap = dflat[doff["ecol"]:doff["ecol"] + 128 * T_tot].rearrange(
        "(p t) -> p t", p=128)
    gidx_ap = dflat[doff["gidx"]:doff["gidx"] + 16 * T_tot * 8].bitcast(
        i16).rearrange("(p t) -> p t", p=16)
    snorm_ap = dflat[doff["snorm"]:doff["snorm"] + 128 * GROUPS].rearrange(
        "(p t) -> p t", p=128)
    bshard_row = doff["bshard"] // 128
    y_d = nc.dram_tensor("y", [GPC, 1], f32, kind="ExternalOutput")
    DBG = int(os.environ.get("KERNEL_DEBUG", "0"))
    dbg_d = nc.dram_tensor("dbg", [128, 4096], f32, kind="ExternalOutput") if DBG else None

    with tile.TileContext(nc) as tc:
        with tc.tile_pool(name="dram", bufs=1, space="DRAM") as dram, \
             tc.tile_pool(name="persist", bufs=1) as pp:
            # DRAM internals
            hb = dram.tile([NSHARD, 128], f32, name="hbounce")
            hfp = [dram.tile([N, 128], f32, name=f"hf{l}") for l in range(5)]
            xf = dram.tile([N, 128], f32, name="xf")
            wfull = dram.tile([struct["wrows"], 128], f32, name="wfull")
            bfull = dram.tile([struct["brows"], 128], bf, name="bfull")

            # replicate the sharded inputs on device (the host link is slow);
            # collectives cannot read IO tensors, so bounce via internal DRAM
            xb = dram.tile([NSHARD, 128], f32, name="xb")
            wb = dram.tile([struct["wrows"] // NCORES, 128], f32, name="wb")
            bb = dram.tile([struct["brows"] // NCORES, 128], bf, name="bb")
            NOCC = bool(int(os.environ.get("KERNEL_NOCC", "0")))  # TimelineSim
            AG = lambda i, o: None if NOCC else nc.gpsimd.collective_compute(
                "AllGather", mybir.AluOpType.bypass,
                replica_groups=[list(range(NCORES))], ins=[i], outs=[o])
            nc.sync.dma_start(wb[:], wshard_d.ap())
            nc.sync.dma_start(
                bb[:], dblob_d.ap()[bshard_row:bshard_row + struct["brows"] // NCORES, :])
            AG(xb.opt(), xf.opt())
            AG(wb.opt(), wfull.opt())
            AG(bb.opt(), bfull.opt())

            def load_w(dst, name, blob=wfull, lay=None):
                row0, P, Cc = (lay or wlay)[name]
                nb = (Cc + 127) // 128
                if nb > 1 and Cc % 128 == 0:
                    # all column blocks in one DMA: blob rows (n p) -> dst (p, n)
                    nc.sync.dma_start(
                        dst[:P, :].rearrange("p (n c) -> p n c", c=128),
                        blob[row0:row0 + nb * P, :].rearrange("(n p) c -> p n c", p=P))
                    return
                for k in range(nb):
                    cw = min(128, Cc - 128 * k)
                    nc.sync.dma_start(dst[:P, 128 * k:128 * k + cw],
                                      blob[row0 + P * k:row0 + P * k + P, 0:cw])

            # SBUF that survives into the head phase
            hT_A = pp.tile([128, 4096], f32, name="hT_A")
            ctxT_bf = pp.tile([128, 4096], bf, name="ctxT_bf")
            ctxnm = pp.tile([128, 4096], bf, name="ctxnm")
            ident_s = pp.tile([128, 128], f32, name="ident_s")
            iota_f = pp.tile([128, 128], f32, name="iota_f")
            iota_p = pp.tile([128, 1], f32, name="iota_p")
            convT = pp.tile([128, 512], bf, name="convT")
            gT = pp.tile([128, GPC], bf, name="gT")
            xtT = pp.tile([128, GPC], bf, name="xtT")
            g2T = pp.tile([128, GPC], bf, name="g2T")
            # iota row 0..127 per partition; iota col = partition index;
            # identity = (row iota == col iota)
            nc.gpsimd.iota(iota_f[:], [[1, 128]], channel_multiplier=0,
                           allow_small_or_imprecise_dtypes=True)
            nc.gpsimd.iota(iota_p[:], [[1, 1]], channel_multiplier=1,
                           allow_small_or_imprecise_dtypes=True)
            nc.vector.tensor_scalar(ident_s[:], iota_f[:], iota_p[:, 0:1], None,
                                    op0=ALU.is_equal)

            # ---------------- GCN layers ----------------
            with tc.tile_pool(name="gcnc", bufs=1) as cp, \
                 tc.tile_pool(name="msgp", bufs=3) as msgp, \
                 tc.tile_pool(name="zp", bufs=3) as zp, \
                 tc.tile_pool(name="aggp", bufs=3, space="PSUM") as aggp, \
                 tc.tile_pool(name="p2p", bufs=2, space="PSUM") as p2p, \
                 tc.tile_pool(name="p3p", bufs=2, space="PSUM") as p3p:
                hT_B = cp.tile([128, 4096], f32, name="hT_B")
                nm0 = cp.tile([128, 4096], f32, name="nm0")
                nm1 = cp.tile([128, 4096], f32, name="nm1")
                gidx_s = cp.tile([128, T_tot * 8], i16, name="gidx_s")
                eoh_s = cp.tile([128, sumw], f32, name="eoh_s")
                soh_s = cp.tile([128, 4096], f32, name="soh_s")
                ecol_b = cp.tile([128, T_tot], bf, name="ecol_b")
                ecol_s = cp.tile([128, T_tot], f32, name="ecol_s")
                ew_b = cp.tile([128, T_tot], bf, name="ew_b")
                ew_s = cp.tile([128, T_tot], f32, name="ew_s")
                snorm_b = cp.tile([128, GROUPS], bf, name="snorm_b")
                snorm_s = cp.tile([128, GROUPS], f32, name="snorm_s")
                Wt_s = cp.tile([128, 768], f32, name="Wt_s")
                bias_s = cp.tile([128, 6], f32, name="bias_s")
                xload = cp.tile([128, GROUPS * 78], i8, name="xload")
                for dst_t, src_ap in [(ecol_b, ecol_ap), (ew_b, ew_ap),
                                      (snorm_b, snorm_ap)]:
                    nc.sync.dma_start(dst_t[:], src_ap)
                nc.vector.tensor_copy(ecol_s[:], ecol_b[:])
                nc.vector.tensor_copy(ew_s[:], ew_b[:])
                nc.vector.tensor_copy(snorm_s[:], snorm_b[:])
                for k in range(8):
                    nc.sync.dma_start(gidx_s[16 * k:16 * (k + 1), :], gidx_ap)
                load_w(Wt_s, "Wt")
                load_w(bias_s, "bias")
                # build the edge scatter one-hots: eoh[:, colo+j] =
                #   (j == ecol[:, fi]) * ew[:, fi]
                for fi, (g, t, off, wid, colo) in enumerate(tiles):
                    nc.vector.tensor_scalar(
                        eoh_s[:, colo:colo + wid], iota_f[:, :wid],
                        ecol_s[:, fi:fi + 1], ew_s[:, fi:fi + 1],
                        op0=ALU.is_equal, op1=ALU.mult)
                # self-loop one-hots: block-diag(snorm)
                for g in range(GROUPS):
                    nc.vector.tensor_scalar(
                        soh_s[:, g * 128:(g + 1) * 128], ident_s[:],
                        snorm_s[:, g:g + 1], None, op0=ALU.mult)
                # node-major x load (int8) -> dequantize to f32 nm0 -> write the
                # f32 shard back to DRAM for the gather-source AllGather.
                # Only the first 78 feature cols exist; the rest are never read.
                nc.sync.dma_start(
                    xload[:].rearrange("p (g c) -> p g c", c=78), x_nm_ap)
                nc.vector.tensor_scalar(
                    nm0[:].rearrange("p (g c) -> p g c", c=128)[:, :, 0:78],
                    xload[:].rearrange("p (g c) -> p g c", c=78),
                    float(struct["xscale"]), None, op0=ALU.mult)
                nc.sync.dma_start(
                    xb[:, 0:78].rearrange("(g p) c -> p g c", p=128),
                    nm0[:].rearrange("p (g c) -> p g c", c=128)[:, :, 0:78])
                NLAYERS = int(os.environ.get("KERNEL_NLAYERS", "6"))
                for l in range(NLAYERS):
                    d_in, d_out = GCN_DIMS[l]
                    gsrc = xf[:] if l == 0 else hfp[l - 1][:]
                    nm_cur = nm0 if l % 2 == 0 else nm1
                    nm_nxt = nm1 if l % 2 == 0 else nm0
                    hT_out = hT_B if l % 2 == 0 else hT_A
                    hT_res = hT_B  # residual source for l in (1, 3)

                    # gather calls
                    EDGES = bool(int(os.environ.get("KERNEL_EDGES", "1")))
                    nchunks = (T_tot + CHUNK_TILES - 1) // CHUNK_TILES if EDGES else 0
                    msgs = []
                    for ci in range(nchunks):
                        t0 = ci * CHUNK_TILES
                        ntc = min(CHUNK_TILES, T_tot - t0)
                        m = msgp.tile([128, CHUNK_TILES, 128], f32, name=f"msg_{l}_{ci}", tag="msg")
                        nc.gpsimd.dma_gather(
                            m[:, :ntc, :], gsrc, gidx_s[:, t0 * 8:(t0 + ntc) * 8],
                            ntc * 128, ntc * 128, 128, single_packet=False)
                        msgs.append(m)

                    fi = 0
                    for g4 in range(0, GROUPS, 4):
                        # 4 groups share one PSUM tile / z copy / W matmul / act
                        agg = aggp.tile([128, 512], f32, name=f"agg_{l}_{g4}", tag="agg")
                        for gi in range(4):
                            g = g4 + gi
                            co = gi * 128
                            nc.tensor.matmul(agg[:d_in, co:co + 128],
                                             nm_cur[:, g * 128:g * 128 + d_in],
                                             soh_s[:, g * 128:(g + 1) * 128],
                                             start=True, stop=not EDGES, skip_group_check=True)
                            ntg = struct["NT"][g] if EDGES else 0
                            for t in range(ntg):
                                (gg, tt, off, wid, colo) = tiles[fi]
                                assert gg == g and tt == t
                                m = msgs[fi // CHUNK_TILES]
                                nc.tensor.matmul(agg[:d_in, co + off:co + off + wid],
                                                 m[:, fi % CHUNK_TILES, :d_in],
                                                 eoh_s[:, colo:colo + wid],
                                                 start=False, stop=(t == ntg - 1),
                                                 skip_group_check=True)
                                fi += 1
                        z = zp.tile([128, 512], f32, name=f"z_{l}_{g4}", tag="z")
                        nc.scalar.copy(z[:d_in, :], agg[:d_in, :])
                        p2 = p2p.tile([128, 512], f32, name=f"p2_{l}_{g4}", tag="p2")
                        nc.tensor.matmul(p2[:d_out, :], Wt_s[:d_in, l * 128:l * 128 + d_out],
                                         z[:d_in, :], start=True, stop=True,
                                         skip_group_check=True)
                        cs4 = slice(g4 * 128, (g4 + 4) * 128)
                        nc.scalar.activation(hT_out[:d_out, cs4], p2[:d_out, :], AF.Relu,
                                             bias=bias_s[:d_out, l:l + 1])
                        if l == 1:
                            nc.vector.tensor_add(hT_out[:78, cs4], hT_out[:78, cs4],
                                                 hT_res[:78, cs4])
                        if l == 3:
                            nc.vector.tensor_add(hT_out[:88, cs4], hT_out[:88, cs4],
                                                 hT_res[:88, cs4])
                        for gi in range(4):
                            g = g4 + gi
                            p3 = p3p.tile([128, 128], f32, name=f"p3_{l}_{g}", tag="p3")
                            nc.tensor.transpose(p3[:, :d_out], hT_out[:d_out, g * 128:(g + 1) * 128],
                                                ident_s[:d_out, :d_out])
                            if l < 5:
                                nc.scalar.copy(nm_nxt[:, g * 128:g * 128 + d_out], p3[:, :d_out])
                            else:
                                nc.scalar.copy(ctxnm[:, g * 128:(g + 1) * 128], p3[:, :128])
                        if l < 5:
                            nc.sync.dma_start(
                                hb[g4 * 128:(g4 + 4) * 128, :].rearrange(
                                    "(g p) c -> p g c", p=128),
                                nm_nxt[:, g4 * 128:(g4 + 4) * 128].rearrange(
                                    "p (g c) -> p g c", c=128))
                    if l < 5 and int(os.environ.get("KERNEL_AG", "1")) and not NOCC:
                        nc.gpsimd.collective_compute(
                            "AllGather", mybir.AluOpType.bypass,
                            replica_groups=[list(range(NCORES))],
                            ins=[hb.opt()], outs=[hfp[l].opt()])
                if DBG == 1:
                    lastL = int(os.environ.get("KERNEL_NLAYERS", "6")) - 1
                    hT_last = hT_B if lastL % 2 == 0 else hT_A
                    nc.sync.dma_start(dbg_d.ap(), hT_last[:])
                elif DBG == 2:
                    lastL = int(os.environ.get("KERNEL_NLAYERS", "6")) - 1
                    nm_last = nm1 if lastL % 2 == 0 else nm0
                    nc.sync.dma_start(dbg_d.ap(), nm_last[:])

            hT6 = hT_A  # layer 5 output

            HEAD = bool(int(os.environ.get("KERNEL_HEAD", "1")))
            if not HEAD:
                with tc.tile_pool(name="dummy", bufs=1) as dp:
                    yDm = dp.tile([1, GPC], f32, name="yDm")
                    nc.vector.memset(yDm[:1, :], 0.0)
                    nc.sync.dma_start(y_d.ap().opt(), yDm[:1, :])


            # ---------------- head: passes A+B ----------------
            if HEAD:
             with tc.tile_pool(name="hcA", bufs=1) as hc, \
                 tc.tile_pool(name="hatt", bufs=1) as hatt:
                Wud_s = hc.tile([128, 256], bf, name="Wud_s")
                sumsel_s = hc.tile([128, 16 * GPC], bf, name="sumsel_s")
                bsel_s = hc.tile([16, 128 * GPC], f32, name="bsel_s")
                convw_s = hc.tile([128, 2048], bf, name="convw_s")
                convb_s = hc.tile([32, 1], f32, name="convb_s")
                qemb_s = hc.tile([32, 128], bf, name="qemb_s")
                for name, dst_t in [("qemb", qemb_s), ("W_updown", Wud_s),
                                    ("sumsel", sumsel_s), ("convw", convw_s)]:
                    load_w(dst_t, name, blob=bfull, lay=blay)
                load_w(bsel_s, "bsel")
                load_w(convb_s, "convb")
                # bf16 copy of transposed ctx for the scores matmuls
                nc.vector.tensor_copy(ctxT_bf[:], hT6[:])

                # tanh(attention) for all graphs: [seq%128, seq-tile, graph, l]
                attF = hatt.tile([128, 8, GPC, 128], bf, name="attF")

                # expT/qTb live only through passes A+B1; freed before the conv
                with tc.tile_pool(name="hbig", bufs=1) as hbig:
                 expT = hbig.tile([128, 32 * 1024], bf, name="expT")
                 qTb = hbig.tile([128, GPC * 1024], bf, name="qTb")
                 recipS = hbig.tile([16, 1024], f32, name="recipS")
                 recip_r = hbig.tile([16, 1024], f32r, name="recip_r")

                 # pass A
                 with tc.tile_pool(name="hwA", bufs=2) as hw, \
                     tc.tile_pool(name="pqA", bufs=1, space="PSUM") as pqA, \
                     tc.tile_pool(name="psc", bufs=2, space="PSUM") as psc, \
                     tc.tile_pool(name="psum_sums", bufs=1, space="PSUM") as pss:
                    sums_ps = pss.tile([16, 1024], f32, name="sums_ps")
                    nc.vector.memset(qTb[:], 0.0)
                    for b in range(GPC):
                        # one-hot of this graph's targets: toht[p, j] = (tgt[b, j] == p);
                        # the DMA replicates the DRAM row across 32 partitions
                        tgtb = hw.tile([32, SEQ], bf, name=f"tgtb_{b}", tag="tgtb")
                        nc.sync.dma_start(
                            tgtb[:], dflat[doff["tgt"] + b * SEQ:doff["tgt"] + (b + 1) * SEQ]
                            .rearrange("(x c) -> x c", x=1).broadcast_to([32, SEQ]))
                        toht = hw.tile([32, SEQ], bf, name=f"toh_{b}", tag="toh")
                        nc.vector.tensor_scalar(
                            toht[:], tgtb[:], iota_p[:32, 0:1], None, op0=ALU.is_equal)
                        qp = pqA.tile([128, 1024], f32, name=f"qp_{b}", tag="qp")
                        for cs, cw in ((0, 512), (512, 488)):
                            nc.tensor.matmul(qp[:, cs:cs + cw],
                                             qemb_s[:26, :],
                                             toht[:26, cs:cs + cw],
                                             start=True, stop=True, skip_group_check=True)
                        qbsl = qTb[:, b * 1024:(b + 1) * 1024]
                        nc.scalar.copy(qbsl[:, :1000], qp[:, :1000])
                        for h in range(2):
                            sp = psc.tile([128, 1024], f32, name=f"sp_{b}_{h}", tag="sp")
                            for cs, cw in ((0, 512), (512, 488)):
                                nc.tensor.matmul(
                                    sp[:, cs:cs + cw],
                                    ctxT_bf[:, 256 * b + 128 * h:256 * b + 128 * h + 128],
                                    qbsl[:, cs:cs + cw],
                                    start=True, stop=True, skip_group_check=True)
                            esl = expT[:, (2 * b + h) * 1024:(2 * b + h) * 1024 + 1000]
                            nc.scalar.activation(esl, sp[:, :1000], AF.Exp)
                            for cs, cw in ((0, 512), (512, 488)):
                                nc.tensor.matmul(
                                    sums_ps[:16, cs:cs + cw],
                                    sumsel_s[:, 16 * b:16 * b + 16],
                                    expT[:, (2 * b + h) * 1024 + cs:(2 * b + h) * 1024 + cs + cw],
                                    start=(b == 0 and h == 0), stop=(b == GPC - 1 and h == 1),
                                    skip_group_check=True)
                    nc.vector.reciprocal(recipS[:16, :1000], sums_ps[:16, :1000])
                    nc.gpsimd.dma_start(recip_r[:16, :1000], recipS[:16, :1000])

                 # pass B1: attention mix + tanh, staged into attF
                 with tc.tile_pool(name="hwB", bufs=2) as hw, \
                     tc.tile_pool(name="prb", bufs=1, space="PSUM") as prb, \
                     tc.tile_pool(name="pmix", bufs=1, space="PSUM") as pmix, \
                     tc.tile_pool(name="patt", bufs=2, space="PSUM") as patt:
                        for b in range(GPC):
                            rb = prb.tile([128, 1024], f32, name=f"rb_{b}", tag="rb")
                            for cs, cw in ((0, 512), (512, 488)):
                                nc.tensor.matmul(rb[:, cs:cs + cw],
                                                 bsel_s[:, 128 * b:128 * (b + 1)].bitcast(f32r),
                                                 recip_r[:16, cs:cs + cw],
                                                 start=True, stop=True, skip_group_check=True)
                            mixp = pmix.tile([128, 1024], f32, name=f"mixp_{b}", tag="mixp")
                            for h in range(2):
                                for cs, cw in ((0, 512), (512, 488)):
                                    nc.tensor.matmul(
                                        mixp[:, cs:cs + cw],
                                        ctxnm[:, (2 * b + h) * 128:(2 * b + h + 1) * 128],
                                        expT[:, (2 * b + h) * 1024 + cs:(2 * b + h) * 1024 + cs + cw],
                                        start=(h == 0), stop=(h == 1), skip_group_check=True)
                            rbS = hw.tile([128, 1024], f32, name=f"rbS_{b}", tag="rbS")
                            nc.scalar.copy(rbS[:, :1000], rb[:, :1000])
                            catU = hw.tile([128, 1024], bf, name=f"catU_{b}", tag="catU")
                            nc.vector.tensor_mul(catU[:, :1000], mixp[:, :1000], rbS[:, :1000])
                            nc.vector.memset(catU[:, 1000:1024], 0.0)
                            for qt in range(8):
                                ap_ = patt.tile([128, 128], f32, name=f"attp_{b}_{qt}", tag="attp")
                                nc.tensor.matmul(ap_[:, :], catU[:, 128 * qt:128 * (qt + 1)],
                                                 Wud_s[:, 0:128], start=True, stop=False,
                                                 skip_group_check=True)
                                nc.tensor.matmul(ap_[:, :],
                                                 qTb[:, b * 1024 + 128 * qt:b * 1024 + 128 * (qt + 1)],
                                                 Wud_s[:, 128:256], start=False, stop=True,
                                                 skip_group_check=True)
                                nc.scalar.activation(attF[:, qt, b, :], ap_[:, :], AF.Tanh)
                        if DBG == 3:
                            nc.sync.dma_start(dbg_d.ap()[0:16, 1024:2024], recipS[:16, :1000])
                            nc.gpsimd.dma_start(dbg_d.ap()[:, 2048:3048], expT[:, 0:1000])
                            nc.gpsimd.dma_start(dbg_d.ap()[:, 3048:4048], qTb[:, 0:1000])

                # hbig (expT/qTb) is freed here; pass B2: conv batched over
                # 4 graphs per PSUM accumulation chain
                with tc.tile_pool(name="hwB2", bufs=2) as hwc, \
                     tc.tile_pool(name="pconv", bufs=2, space="PSUM") as pconv, \
                     tc.tile_pool(name="pct", bufs=2, space="PSUM") as pct:
                    for b4 in range(0, GPC, 4):
                        cps = pconv.tile([32, 4 * 121], f32, name=f"cps_{b4}", tag="cps")
                        cps_v = cps[:32, :].rearrange("p (g l) -> p g l", l=121)
                        for k in range(8):
                            for it in range(8):
                                nc.tensor.matmul(
                                    cps_v,
                                    convw_s[:, (k * 8 + it) * 32:(k * 8 + it) * 32 + 32],
                                    attF[:, it, b4:b4 + 4, k:k + 121],
                                    start=(k == 0 and it == 0), stop=(k == 7 and it == 7),
                                    skip_group_check=True)
                        convS = hwc.tile([32, 4 * 121], f32, name=f"convS_{b4}", tag="convS")
                        nc.scalar.activation(convS[:32, :], cps[:32, :], AF.Identity,
                                             bias=convb_s[:32, 0:1])
                        for gi in range(4):
                            b = b4 + gi
                            ct = pct.tile([128, 32], f32, name=f"ct_{b}", tag="ct")
                            nc.tensor.transpose(ct[:121, :32],
                                                convS[:32, gi * 121:(gi + 1) * 121],
                                                ident_s[:32, :32])
                            nc.scalar.copy(convT[:121, 32 * b:32 * (b + 1)], ct[:121, :32])

            # ---------------- head: pooling + final MLPs ----------------
            if HEAD:
             with tc.tile_pool(name="hcB", bufs=1) as hc2, \
                 tc.tile_pool(name="hwC", bufs=1) as hw2, \
                 tc.tile_pool(name="pf", bufs=1, space="PSUM") as pf:
                fc1xtw_s = hc2.tile([128, 4096], bf, name="fc1xtw_s")
                fc1xtb_s = hc2.tile([128, 1], f32, name="fc1xtb_s")
                fcg1w_s = hc2.tile([128, 1024], bf, name="fcg1w_s")
                fcg1b_s = hc2.tile([128, 8], f32, name="fcg1b_s")
                fcg2w_s = hc2.tile([128, 1024], bf, name="fcg2w_s")
                fcg2b_s = hc2.tile([128, 1], f32, name="fcg2b_s")
                fc1w_s = hc2.tile([128, 2048], bf, name="fc1w_s")
                fc1b_s = hc2.tile([128, 8], f32, name="fc1b_s")
                fc2w_s = hc2.tile([128, 4096], bf, name="fc2w_s")
                fc2b_s = hc2.tile([128, 4], f32, name="fc2b_s")
                outw_s = hc2.tile([128, 4], f32, name="outw_s")
                for name, dst_t in [("fc1xtb", fc1xtb_s), ("fcg1b", fcg1b_s),
                                    ("fcg2b", fcg2b_s), ("fc1b", fc1b_s),
                                    ("fc2b", fc2b_s), ("outw", outw_s)]:
                    load_w(dst_t, name)
                for name, dst_t in [("fc1xtw", fc1xtw_s), ("fcg1w", fcg1w_s),
                                    ("fcg2w", fcg2w_s), ("fc1w", fc1w_s),
                                    ("fc2w", fc2w_s)]:
                    load_w(dst_t, name, blob=bfull, lay=blay)

                # g pooling + graph MLP
                hT6_v = hT6[:].rearrange("p (b n) -> p b n", n=256)
                nc.vector.tensor_reduce(gT[:, :GPC], hT6_v, mybir.AxisListType.X,
                                        mybir.AluOpType.max)
                g1_ps = pf.tile([128, 128], f32, name="g1_ps")
                for j in range(8):
                    nc.tensor.matmul(g1_ps[:, 16 * j:16 * (j + 1)],
                                     fcg1w_s[:, 128 * j:128 * (j + 1)], gT[:, :GPC],
                                     start=True, stop=True, skip_group_check=True)
                g1T = hw2.tile([128, 128], bf, name="g1T")
                for j in range(8):
                    nc.scalar.activation(g1T[:, 16 * j:16 * (j + 1)],
                                         g1_ps[:, 16 * j:16 * (j + 1)], AF.Relu,
                                         bias=fcg1b_s[:, j:j + 1])
                g2_ps = pf.tile([128, GPC], f32, name="g2_ps")
                for j in range(8):
                    nc.tensor.matmul(g2_ps[:, :], fcg2w_s[:, 128 * j:128 * (j + 1)],
                                     g1T[:, 16 * j:16 * (j + 1)],
                                     start=(j == 0), stop=(j == 7), skip_group_check=True)
                nc.scalar.activation(g2T[:, :], g2_ps[:, :], AF.Identity,
                                     bias=fcg2b_s[:, 0:1])

                # fc1xt
                convT_v = convT[:121, :].rearrange("p (b o) -> p b o", o=32)
                xt_ps = pf.tile([128, GPC], f32, name="xt_ps")
                for oc in range(32):
                    nc.tensor.matmul(xt_ps[:, :], fc1xtw_s[:121, 128 * oc:128 * (oc + 1)],
                                     convT_v[:, :, oc], start=(oc == 0), stop=(oc == 31),
                                     skip_group_check=True)
                nc.scalar.activation(xtT[:, :], xt_ps[:, :], AF.Identity,
                                     bias=fc1xtb_s[:, 0:1])

                h1_ps = pf.tile([128, 128], f32, name="h1_ps")
                for j in range(8):
                    nc.tensor.matmul(h1_ps[:, 16 * j:16 * (j + 1)],
                                     fc1w_s[:, 128 * j:128 * (j + 1)], g2T[:, :],
                                     start=True, stop=False, skip_group_check=True)
                    nc.tensor.matmul(h1_ps[:, 16 * j:16 * (j + 1)],
                                     fc1w_s[:, 1024 + 128 * j:1024 + 128 * (j + 1)],
                                     xtT[:, :], start=False, stop=True,
                                     skip_group_check=True)
                h1T = hw2.tile([128, 128], bf, name="h1T")
                for j in range(8):
                    nc.scalar.activation(h1T[:, 16 * j:16 * (j + 1)],
                                         h1_ps[:, 16 * j:16 * (j + 1)], AF.Relu,
                                         bias=fc1b_s[:, j:j + 1])
                h2_ps = pf.tile([128, 64], f32, name="h2_ps")
                for j in range(4):
                    for k in range(8):
                        nc.tensor.matmul(
                            h2_ps[:, 16 * j:16 * (j + 1)],
                            fc2w_s[:, 128 * (4 * k + j):128 * (4 * k + j + 1)],
                            h1T[:, 16 * k:16 * (k + 1)],
                            start=(k == 0), stop=(k == 7), skip_group_check=True)
                h2T = hw2.tile([128, 64], f32, name="h2T")
                for j in range(4):
                    nc.scalar.activation(h2T[:, 16 * j:16 * (j + 1)],
                                         h2_ps[:, 16 * j:16 * (j + 1)], AF.Relu,
                                         bias=fc2b_s[:, j:j + 1])
                y_ps = pf.tile([1, GPC], f32, name="y_ps")
                for k in range(4):
                    nc.tensor.matmul(y_ps[:1, :], outw_s[:, k:k + 1],
                                     h2T[:, 16 * k:16 * (k + 1)],
                                     start=(k == 0), stop=(k == 3), skip_group_check=True)
                yS = hw2.tile([1, GPC], f32, name="yS")
                nc.scalar.activation(yS[:1, :], y_ps[:1, :], AF.Identity,
                                     bias=float(outb))
                nc.sync.dma_start(y_d.ap().opt(), yS[:1, :])
                if DBG == 3:
                    nc.sync.dma_start(dbg_d.ap()[:, 0:512], convT[:])
                    nc.sync.dma_start(dbg_d.ap()[:, 512:528], gT[:])
                    nc.sync.dma_start(dbg_d.ap()[:, 528:544], g2T[:])
                    nc.sync.dma_start(dbg_d.ap()[:, 544:560], xtT[:])

    nc.compile()
    return nc


def kernel(**inputs) -> np.ndarray:
    from concourse.bass_utils import run_bass_kernel_spmd
    import jax
    try:
        # cache the XLA executable on disk: run_bass_via_pjrt re-jits a fresh
        # closure per call, and without this every call repays ~0.9s of
        # BIR verify + NEFF wrap even with a warm neuronxcc cache
        jax.config.update("jax_compilation_cache_dir", "/tmp/jaxcache")
        jax.config.update("jax_persistent_cache_min_compile_time_secs", 0.0)
        jax.config.update("jax_persistent_cache_min_entry_size_bytes", -1)
    except Exception as e:
        print(f"jax compilation cache unavailable: {e}", flush=True)

    t0 = time.time()
    struct, per_core, outb = _host_prep(inputs)
    t1 = time.time()
    nc = _build(struct, outb)
    t2 = time.time()
    print(f"host_prep {t1 - t0:.3f}s  build+compile {t2 - t1:.3f}s", flush=True)

    in_maps = [dict(per_core[c]) for c in range(NCORES)]

    trace = bool(int(os.environ.get("KERNEL_TRACE", "0")))
    t_run = time.time()
    try:
        res = run_bass_kernel_spmd(nc, in_maps, core_ids=list(range(NCORES)), trace=False)
    except Exception as e:  # transient NRT device errors observed under axon; retry once
        print(f"spmd attempt 1 failed ({type(e).__name__}); retrying once", flush=True)
        time.sleep(5)
        res = run_bass_kernel_spmd(nc, in_maps, core_ids=list(range(NCORES)), trace=False)
    print(f"spmd wall #1 (compile+transfer+exec): {time.time() - t_run:.3f}s", flush=True)
    if trace:
        # second run reuses the jit/NEFF cache: wall is transfer+exec only
        best = None
        for _ in range(int(os.environ.get("KERNEL_REPS", "7"))):
            t_run = time.time()
            res = run_bass_kernel_spmd(nc, in_maps, core_ids=list(range(NCORES)), trace=False)
            wall2 = time.time() - t_run
            print(f"spmd wall (transfer+exec): {wall2:.3f}s", flush=True)
            best = wall2 if best is None else min(best, wall2)
        print(f"HW exec time: {int(best * 1e9)} ns (wall-clock upper bound; "
              f"NTFF profiling unavailable under this axon tunnel)")
    if int(os.environ.get("KERNEL_DEBUG", "0")):
        np.save("/tmp/dbg.npy", np.stack([res.results[c]["dbg"] for c in range(NCORES)]))
    out = np.concatenate([res.results[c]["y"] for c in range(NCORES)], axis=0)
    return out.astype(np.float32)


# revision 71
# speedup vs baseline: 1.0278x; 1.0278x over previous
"""Trainium2 Bass kernel for nn_AttnGCNNet (GCN message passing + attention + conv).

Strategy (8 NeuronCores, SPMD single NEFF):
  - Nodes/graphs sharded by contiguous ranges: core c owns nodes [c*4096, (c+1)*4096)
    = 16 whole graphs. Edges sharded by dst node.
  - GCN layer: agg = A_norm @ h done as: dma_gather of h rows (by edge src, dst-sorted)
    from a replicated HBM copy of h, then PE one-hot scatter matmuls into PSUM
    (transposed layout [d, nodes]); self-loop term via block-diag one-hot from an
    SBUF node-major copy; then (agg)@W + b + relu on PE/ACT; node-major write-back via
    PE transpose; AllGather replicates the new h shard to all cores' HBM.
  - Attention/conv/MLP head: per-graph matmuls in transposed layouts; softmax along
    free axis with PE-computed row sums; conv as 64 accumulating matmuls.
  - Host<->device transfer minimized (the axon tunnel is ~60 MB/s): x is sent as
    bf16 [4096,78] per-core shards, converted to f32 and AllGathered on device;
    the edge/self/target one-hots (eoh/soh/toh) and the identity are built on
    device from compact index/weight arrays (iota + fused compare*weight); the
    replicated weights travel as 1/8 shards of two packed blobs (f32 + bf16,
    head MLP weights in bf16) and are AllGathered on device.
  - Per-instruction overhead in this environment is ~10us, so instruction count
    is minimized: GCN post-ops batched 4 groups per PSUM tile, conv batched 4
    graphs per accumulation chain, blob loads one DMA per tensor.
  - kernel() enables the jax persistent compilation cache: run_bass_via_pjrt
    re-jits a fresh closure per call, and without the cache every call repays
    ~0.9s of BIR verify + NEFF wrap even with a warm neuronxcc cache.
"""
import os
import time
import numpy as np

B, N_PER, E, SEQ, VOCAB, D = 128, 256, 262144, 1000, 26, 128
N = B * N_PER
GCN_DIMS = [(78, 78), (78, 88), (88, 88), (88, 118), (118, 118), (118, 128)]
NCORES = 8
NSHARD = N // NCORES       # 4096
GROUPS = NSHARD // 128     # 32
GPC = B // NCORES          # 16 graphs per core
CHUNK_TILES = 48           # edge tiles per dma_gather call


# ---------------------------------------------------------------- host packing
def _pack_edges(src, dst, enorm):
    """Shared-structure edge tiling. Returns (structure, per-core arrays)."""
    per_core = []
    for c in range(NCORES):
        lo = c * NSHARD
        sel = np.nonzero((dst >= lo) & (dst < lo + NSHARD))[0]
        ed = (dst[sel] - lo).astype(np.int64)
        order = np.argsort(ed, kind="stable")
        per_core.append((src[sel][order], ed[order], enorm[sel][order]))

    # group tile counts (shared across cores)
    NT = []
    cge = [[None] * GROUPS for _ in range(NCORES)]
    for g in range(GROUPS):
        cnt = 1
        for c in range(NCORES):
            es, ed, ew = per_core[c]
            m = (ed >= g * 128) & (ed < (g + 1) * 128)
            cge[c][g] = (es[m], ed[m] - g * 128, ew[m])
            cnt = max(cnt, (len(ew[m]) + 127) // 128)
        NT.append(cnt)

    # union windows per (g, t)
    tiles = []  # flat list of (g, t, off, wid, col)
    col = 0
    for g in range(GROUPS):
        for t in range(NT[g]):
            lo_w, hi_w = 128, -1
            for c in range(NCORES):
                ged = cge[c][g][1][t * 128:(t + 1) * 128]
                if len(ged):
                    lo_w = min(lo_w, int(ged.min()))
                    hi_w = max(hi_w, int(ged.max()))
            if hi_w < 0:
                lo_w, hi_w = 0, 0
            wid = hi_w - lo_w + 1
            tiles.append((g, t, lo_w, wid, col))
            col += wid
    sumw = col
    T_tot = len(tiles)

    gidx, ecols, ews = [], [], []
    for c in range(NCORES):
        all_idx = np.zeros(T_tot * 128, dtype=np.int16)
        ecol = np.zeros((128, T_tot), dtype=np.uint8)
        ew = np.zeros((128, T_tot), dtype=np.float32)
        for fi, (g, t, off, wid, colo) in enumerate(tiles):
            ges, ged, gew = cge[c][g]
            sl = slice(t * 128, (t + 1) * 128)
            e_s, e_d, e_w = ges[sl], ged[sl], gew[sl]
            n = len(e_s)
            all_idx[fi * 128:fi * 128 + n] = e_s.astype(np.int16)
            if n:
                ecol[:n, fi] = (e_d - off).astype(np.uint8)
                ew[:n, fi] = e_w
        C = T_tot * 8
        wrapped = all_idx.reshape(C, 16).T  # [16, C]
        gidx.append(np.ascontiguousarray(wrapped))
        ecols.append(ecol)
        ews.append(ew)
    return dict(NT=NT, tiles=tiles, sumw=sumw, T_tot=T_tot), gidx, ecols, ews


def _blob_pack(tensors):
    """Pack [P<=128, C] matrices into a stack of [P,128] blocks.

    Returns (blob [R,128], layout name -> (row0, P, C))."""
    layout = {}
    blocks = []
    row = 0
    for name, a in tensors:
        P, Cc = a.shape
        nb = (Cc + 127) // 128
        for k in range(nb):
            blk = np.zeros((P, 128), a.dtype)
            cw = min(128, Cc - 128 * k)
            blk[:, :cw] = a[:, 128 * k:128 * k + cw]
            blocks.append(blk)
        layout[name] = (row, P, Cc)
        row += nb * P
    if row % NCORES:
        blocks.append(np.zeros((NCORES - row % NCORES, 128), tensors[0][1].dtype))
    blob = np.ascontiguousarray(np.concatenate(blocks, axis=0))
    assert blob.shape[0] % NCORES == 0, blob.shape
    return blob, layout


def _host_prep(inputs):
    import ml_dtypes
    bf16 = np.dtype(ml_dtypes.bfloat16)
    inp = {k: np.asarray(v) for k, v in inputs.items()}
    x = inp["x"].astype(np.float32)
    src = inp["edge_index"][0].astype(np.int64)
    dst = inp["edge_index"][1].astype(np.int64)
    target = inp["target"].astype(np.int64)

    deg = 1.0 + np.bincount(dst, minlength=N).astype(np.float32)
    dis = (1.0 / np.sqrt(deg)).astype(np.float32)
    enorm = (dis[src] * dis[dst]).astype(np.float32)
    snorm = (dis * dis).astype(np.float32)

    struct, gidx, ecols, ews = _pack_edges(src, dst, enorm)

    # per-core snorm shard in [128 part, 32 group] layout
    snorms = []
    for c in range(NCORES):
        s = snorm[c * NSHARD:(c + 1) * NSHARD].reshape(GROUPS, 128).T
        snorms.append(np.ascontiguousarray(s.astype(np.float32)))

    # GCN weights [128, 6*128], biases [128, 6]
    Wt = np.zeros((128, 6 * 128), dtype=np.float32)
    bias = np.zeros((128, 6), dtype=np.float32)
    for l, (di, do) in enumerate(GCN_DIMS, 0):
        Wt[:di, l * 128:l * 128 + do] = inp["W%d" % (l + 1)].astype(np.float32)
        bias[:do, l] = inp["b%d" % (l + 1)].astype(np.float32)

    qemb = (inp["emb"].astype(np.float32) @ inp["attn_in_w"].astype(np.float32))
    qemb_bf = np.zeros((32, 128), dtype=bf16)
    qemb_bf[:26, :] = qemb.astype(bf16)
    W_updown = np.zeros((128, 256), dtype=bf16)
    W_updown[:, 0:128] = inp["attn_out_w"][:128, :].astype(bf16)
    W_updown[:, 128:256] = inp["attn_out_w"][128:, :].astype(bf16)

    # target indices per core [16, 1000] (one-hots are built on device)
    tgts = []
    for c in range(NCORES):
        t = target[c * GPC:(c + 1) * GPC].astype(np.uint8)
        tgts.append(np.ascontiguousarray(t))

    sumsel = np.zeros((128, 16 * GPC), dtype=bf16)
    for b in range(GPC):
        sumsel[:, 16 * b + b] = 1.0
    bsel = np.zeros((16, 128 * GPC), dtype=np.float32)
    for b in range(GPC):
        bsel[b, 128 * b:128 * (b + 1)] = 1.0

    # conv weights [128, 8*8*32] bf16 (k-major, then ic-tile), zero-padded ic
    cw = inp["conv_w"].astype(np.float32)  # [32, 1000, 8]
    convw = np.zeros((128, 8 * 8 * 32), dtype=np.float32)
    for k in range(8):
        for it in range(8):
            icn = min(128, 1000 - it * 128)
            blk = cw[:, it * 128:it * 128 + icn, k]  # [32, icn]
            convw[:icn, (k * 8 + it) * 32:(k * 8 + it) * 32 + 32] = blk.T
    convw = convw.astype(bf16)
    convb = np.zeros((32, 1), dtype=np.float32)
    convb[:, 0] = inp["conv_b"].astype(np.float32)

    f = lambda a: a.astype(np.float32)
    fc1xtw = np.zeros((128, 32 * 128), dtype=np.float32)
    for oc in range(32):
        fc1xtw[:121, oc * 128:(oc + 1) * 128] = f(inp["fc1xt_w"])[oc * 121:(oc + 1) * 121, :]
    fc1xtb = f(inp["fc1xt_b"]).reshape(128, 1)

    fcg1w = f(inp["fcg1_w"])                    # [128, 1024]
    fcg1b = np.stack([f(inp["fcg1_b"])[128 * j:128 * (j + 1)] for j in range(8)], axis=1)
    fcg2w = np.concatenate([f(inp["fcg2_w"])[128 * j:128 * (j + 1), :] for j in range(8)], axis=1)  # [128, 1024]
    fcg2b = f(inp["fcg2_b"]).reshape(128, 1)
    fc1w = np.concatenate([f(inp["fc1_w"])[128 * k:128 * (k + 1), :] for k in range(2)], axis=1)  # [128, 2048]
    fc1b = np.stack([f(inp["fc1_b"])[128 * j:128 * (j + 1)] for j in range(8)], axis=1)  # [128, 8]
    fc2w = np.concatenate([f(inp["fc2_w"])[128 * k:128 * (k + 1), :] for k in range(8)], axis=1)  # [128, 4096]
    fc2b = np.stack([f(inp["fc2_b"])[128 * j:128 * (j + 1)] for j in range(4)], axis=1)  # [128, 4]
    outw = np.concatenate([f(inp["out_w"])[128 * k:128 * (k + 1), :] for k in range(4)], axis=1)  # [128, 4]
    outb = float(np.asarray(inp["out_b"]).reshape(-1)[0])

    wblob, wlay = _blob_pack([
        ("Wt", Wt), ("bias", bias), ("fc1xtb", fc1xtb),
        ("fcg1b", fcg1b), ("fcg2b", fcg2b),
        ("fc1b", fc1b), ("fc2b", fc2b),
        ("outw", outw), ("convb", convb), ("bsel", bsel),
    ])
    bblob, blay = _blob_pack([
        ("qemb", qemb_bf), ("W_updown", W_updown), ("sumsel", sumsel),
        ("convw", convw),
        ("fc1xtw", fc1xtw.astype(bf16)),
        ("fcg1w", fcg1w.astype(bf16)), ("fcg2w", fcg2w.astype(bf16)),
        ("fc1w", fc1w.astype(bf16)), ("fc2w", fc2w.astype(bf16)),
    ])
    struct["wlay"], struct["wrows"] = wlay, wblob.shape[0]
    struct["blay"], struct["brows"] = blay, bblob.shape[0]

    # consolidate all per-core bf16/raw data into one input blob (fewer
    # host->device arrays: per-array transfer overhead is real on this link);
    # x goes separately as int8 (absmax-scaled) to halve its bytes
    xscale = float(np.abs(x).max()) / 127.0
    xq = np.clip(np.round(x / xscale), -127, 127).astype(np.int8)
    per_core = []
    rf, rb = wblob.shape[0] // NCORES, bblob.shape[0] // NCORES
    doff = {}
    for c in range(NCORES):
        parts = [
            ("ew", ews[c].astype(bf16)),
            ("ecol", ecols[c].astype(bf16)),
            ("tgt", tgts[c].astype(bf16)),
            ("gidx", gidx[c].view(bf16)),
            ("snorm", snorms[c].astype(bf16)),
            ("bshard", bblob[c * rb:(c + 1) * rb]),
        ]
        cur, flats = 0, []
        for name, a in parts:
            fl = np.ascontiguousarray(a).reshape(-1)
            assert fl.size % 128 == 0, (name, fl.size)
            doff[name] = cur
            cur += fl.size
            flats.append(fl)
        dblob = np.concatenate(flats).reshape(-1, 128)
        per_core.append(dict(
            dblob=dblob,
            xq=np.ascontiguousarray(xq[c * NSHARD:(c + 1) * NSHARD, :]),
            wshard=np.ascontiguousarray(wblob[c * rf:(c + 1) * rf])))
    struct["doff"], struct["drows"] = doff, per_core[0]["dblob"].shape[0]
    struct["xscale"] = xscale
    return struct, per_core, outb


# ---------------------------------------------------------------- device build
def _build(struct, outb):
    import concourse.bacc as bacc
    import concourse.bass as bass
    import concourse.tile as tile
    import concourse.mybir as mybir

    f32 = mybir.dt.float32
    f32r = mybir.dt.float32r
    bf = mybir.dt.bfloat16
    i16 = mybir.dt.int16
    u8 = mybir.dt.uint8
    AF = mybir.ActivationFunctionType
    ALU = mybir.AluOpType
    tiles, T_tot, sumw = struct["tiles"], struct["T_tot"], struct["sumw"]
    wlay, blay = struct["wlay"], struct["blay"]
    doff = struct["doff"]

    nc = bacc.Bacc("TRN2", target_bir_lowering=False, debug=False, num_devices=NCORES)

    # I/O: one bf16 blob with all per-core data + int8 x + the f32 weight shard
    i8 = mybir.dt.int8
    ein = lambda n, s, d: nc.dram_tensor(n, s, d, kind="ExternalInput")
    dblob_d = ein("dblob", [struct["drows"], 128], bf)
    xq_d = ein("xq", [NSHARD, 78], i8)
    wshard_d = ein("wshard", [struct["wrows"] // NCORES, 128], f32)
    dflat = dblob_d.ap().rearrange("r c -> (r c)")
    x_nm_ap = xq_d.ap().rearrange("(g p) c -> p g c", p=128)
    ew_ap = dflat[doff["ew"]:doff["ew"] + 128 * T_tot].rearrange(
        "(p t) -> p t", p=128)
    ecol_ap = dflat[doff["ecol"]:doff["ecol"] + 128 * T_tot].rearrange(
        "(p t) -> p t", p=128)
    gidx_ap = dflat[doff["gidx"]:doff["gidx"] + 16 * T_tot * 8].bitcast(
        i16).rearrange("(p t) -> p t", p=16)
    snorm_ap = dflat[doff["snorm"]:doff["snorm"] + 128 * GROUPS].rearrange(
        "(p t) -> p t", p=128)
    bshard_row = doff["bshard"] // 128
    y_d = nc.dram_tensor("y", [GPC, 1], f32, kind="ExternalOutput")
    DBG = int(os.environ.get("KERNEL_DEBUG", "0"))
    dbg_d = nc.dram_tensor("dbg", [128, 4096], f32, kind="ExternalOutput") if DBG else None

    with tile.TileContext(nc) as tc:
        with tc.tile_pool(name="dram", bufs=1, space="DRAM") as dram, \
             tc.tile_pool(name="persist", bufs=1) as pp:
            # DRAM internals
            hb = dram.tile([NSHARD, 128], f32, name="hbounce")
            hfp = [dram.tile([N, 128], f32, name=f"hf{l}") for l in range(5)]
            xf = dram.tile([N, 128], f32, name="xf")
            wfull = dram.tile([struct["wrows"], 128], f32, name="wfull")
            bfull = dram.tile([struct["brows"], 128], bf, name="bfull")

            # replicate the sharded inputs on device (the host link is slow);
            # collectives cannot read IO tensors, so bounce via internal DRAM
            xb = dram.tile([NSHARD, 128], f32, name="xb")
            wb = dram.tile([struct["wrows"] // NCORES, 128], f32, name="wb")
            bb = dram.tile([struct["brows"] // NCORES, 128], bf, name="bb")
            NOCC = bool(int(os.environ.get("KERNEL_NOCC", "0")))  # TimelineSim
            AG = lambda i, o: None if NOCC else nc.gpsimd.collective_compute(
                "AllGather", mybir.AluOpType.bypass,
                replica_groups=[list(range(NCORES))], ins=[i], outs=[o])
            nc.sync.dma_start(wb[:], wshard_d.ap())
            nc.sync.dma_start(
                bb[:], dblob_d.ap()[bshard_row:bshard_row + struct["brows"] // NCORES, :])
            AG(xb.opt(), xf.opt())
            AG(wb.opt(), wfull.opt())
            AG(bb.opt(), bfull.opt())

            def load_w(dst, name, blob=wfull, lay=None):
                row0, P, Cc = (lay or wlay)[name]
                nb = (Cc + 127) // 128
                if nb > 1 and Cc % 128 == 0:
                    # all column blocks in one DMA: blob rows (n p) -> dst (p, n)
                    nc.sync.dma_start(
                        dst[:P, :].rearrange("p (n c) -> p n c", c=128),
                        blob[row0:row0 + nb * P, :].rearrange("(n p) c -> p n c", p=P))
                    return
                for k in range(nb):
                    cw = min(128, Cc - 128 * k)
                    nc.sync.dma_start(dst[:P, 128 * k:128 * k + cw],
                                      blob[row0 + P * k:row0 + P * k + P, 0:cw])

            # SBUF that survives into the head phase
            hT_A = pp.tile([128, 4096], f32, name="hT_A")
            ctxT_bf = pp.tile([128, 4096], bf, name="ctxT_bf")
            ctxnm = pp.tile([128, 4096], bf, name="ctxnm")
            ident_s = pp.tile([128, 128], f32, name="ident_s")
            iota_f = pp.tile([128, 128], f32, name="iota_f")
            iota_p = pp.tile([128, 1], f32, name="iota_p")
            convT = pp.tile([128, 512], bf, name="convT")
            gT = pp.tile([128, GPC], bf, name="gT")
            xtT = pp.tile([128, GPC], bf, name="xtT")
            g2T = pp.tile([128, GPC], bf, name="g2T")
            # iota row 0..127 per partition; iota col = partition index;
            # identity = (row iota == col iota)
            nc.gpsimd.iota(iota_f[:], [[1, 128]], channel_multiplier=0,
                           allow_small_or_imprecise_dtypes=True)
            nc.gpsimd.iota(iota_p[:], [[1, 1]], channel_multiplier=1,
                           allow_small_or_imprecise_dtypes=True)
            nc.vector.tensor_scalar(ident_s[:], iota_f[:], iota_p[:, 0:1], None,
                                    op0=ALU.is_equal)

            # ---------------- GCN layers ----------------
            with tc.tile_pool(name="gcnc", bufs=1) as cp, \
                 tc.tile_pool(name="msgp", bufs=2) as msgp, \
                 tc.tile_pool(name="zp", bufs=3) as zp, \
                 tc.tile_pool(name="aggp", bufs=3, space="PSUM") as aggp, \
                 tc.tile_pool(name="p2p", bufs=2, space="PSUM") as p2p, \
                 tc.tile_pool(name="p3p", bufs=2, space="PSUM") as p3p:
                hT_B = cp.tile([128, 4096], f32, name="hT_B")
                nm0 = cp.tile([128, 4096], f32, name="nm0")
                nm1 = cp.tile([128, 4096], f32, name="nm1")
                gidx_s = cp.tile([128, T_tot * 8], i16, name="gidx_s")
                eoh_s = cp.tile([128, sumw], f32, name="eoh_s")
                soh_s = cp.tile([128, 4096], f32, name="soh_s")
                ecol_b = cp.tile([128, T_tot], bf, name="ecol_b")
                ecol_s = cp.tile([128, T_tot], f32, name="ecol_s")
                ew_b = cp.tile([128, T_tot], bf, name="ew_b")
                ew_s = cp.tile([128, T_tot], f32, name="ew_s")
                snorm_b = cp.tile([128, GROUPS], bf, name="snorm_b")
                snorm_s = cp.tile([128, GROUPS], f32, name="snorm_s")
                Wt_s = cp.tile([128, 768], f32, name="Wt_s")
                bias_s = cp.tile([128, 6], f32, name="bias_s")
                xload = cp.tile([128, GROUPS * 78], i8, name="xload")
                for dst_t, src_ap in [(ecol_b, ecol_ap), (ew_b, ew_ap),
                                      (snorm_b, snorm_ap)]:
                    nc.sync.dma_start(dst_t[:], src_ap)
                nc.vector.tensor_copy(ecol_s[:], ecol_b[:])
                nc.vector.tensor_copy(ew_s[:], ew_b[:])
                nc.vector.tensor_copy(snorm_s[:], snorm_b[:])
                for k in range(8):
                    nc.sync.dma_start(gidx_s[16 * k:16 * (k + 1), :], gidx_ap)
                load_w(Wt_s, "Wt")
                load_w(bias_s, "bias")
                # build the edge scatter one-hots: eoh[:, colo+j] =
                #   (j == ecol[:, fi]) * ew[:, fi]
                for fi, (g, t, off, wid, colo) in enumerate(tiles):
                    nc.vector.tensor_scalar(
                        eoh_s[:, colo:colo + wid], iota_f[:, :wid],
                        ecol_s[:, fi:fi + 1], ew_s[:, fi:fi + 1],
                        op0=ALU.is_equal, op1=ALU.mult)
                # self-loop one-hots: block-diag(snorm)
                for g in range(GROUPS):
                    nc.vector.tensor_scalar(
                        soh_s[:, g * 128:(g + 1) * 128], ident_s[:],
                        snorm_s[:, g:g + 1], None, op0=ALU.mult)
                # node-major x load (int8) -> dequantize to f32 nm0 -> write the
                # f32 shard back to DRAM for the gather-source AllGather.
                # Only the first 78 feature cols exist; the rest are never read.
                nc.sync.dma_start(
                    xload[:].rearrange("p (g c) -> p g c", c=78), x_nm_ap)
                nc.vector.tensor_scalar(
                    nm0[:].rearrange("p (g c) -> p g c", c=128)[:, :, 0:78],
                    xload[:].rearrange("p (g c) -> p g c", c=78),
                    float(struct["xscale"]), None, op0=ALU.mult)
                nc.sync.dma_start(
                    xb[:, 0:78].rearrange("(g p) c -> p g c", p=128),
                    nm0[:].rearrange("p (g c) -> p g c", c=128)[:, :, 0:78])
                NLAYERS = int(os.environ.get("KERNEL_NLAYERS", "6"))
                for l in range(NLAYERS):
                    d_in, d_out = GCN_DIMS[l]
                    gsrc = xf[:] if l == 0 else hfp[l - 1][:]
                    nm_cur = nm0 if l % 2 == 0 else nm1
                    nm_nxt = nm1 if l % 2 == 0 else nm0
                    hT_out = hT_B if l % 2 == 0 else hT_A
                    hT_res = hT_B  # residual source for l in (1, 3)

                    # gather calls
                    EDGES = bool(int(os.environ.get("KERNEL_EDGES", "1")))
                    nchunks = (T_tot + CHUNK_TILES - 1) // CHUNK_TILES if EDGES else 0
                    msgs = []
                    for ci in range(nchunks):
                        t0 = ci * CHUNK_TILES
                        ntc = min(CHUNK_TILES, T_tot - t0)
                        m = msgp.tile([128, CHUNK_TILES, 128], f32, name=f"msg_{l}_{ci}", tag="msg")
                        nc.gpsimd.dma_gather(
                            m[:, :ntc, :], gsrc, gidx_s[:, t0 * 8:(t0 + ntc) * 8],
                            ntc * 128, ntc * 128, 128, single_packet=False)
                        msgs.append(m)

                    fi = 0
                    for g4 in range(0, GROUPS, 4):
                        # 4 groups share one PSUM tile / z copy / W matmul / act
                        agg = aggp.tile([128, 512], f32, name=f"agg_{l}_{g4}", tag="agg")
                        for gi in range(4):
                            g = g4 + gi
                            co = gi * 128
                            nc.tensor.matmul(agg[:d_in, co:co + 128],
                                             nm_cur[:, g * 128:g * 128 + d_in],
                                             soh_s[:, g * 128:(g + 1) * 128],
                                             start=True, stop=not EDGES, skip_group_check=True)
                            ntg = struct["NT"][g] if EDGES else 0
                            for t in range(ntg):
                                (gg, tt, off, wid, colo) = tiles[fi]
                                assert gg == g and tt == t
                                m = msgs[fi // CHUNK_TILES]
                                nc.tensor.matmul(agg[:d_in, co + off:co + off + wid],
                                                 m[:, fi % CHUNK_TILES, :d_in],
                                                 eoh_s[:, colo:colo + wid],
                                                 start=False, stop=(t == ntg - 1),
                                                 skip_group_check=True)
                                fi += 1
                        z = zp.tile([128, 512], f32, name=f"z_{l}_{g4}", tag="z")
                        nc.scalar.copy(z[:d_in, :], agg[:d_in, :])
                        p2 = p2p.tile([128, 512], f32, name=f"p2_{l}_{g4}", tag="p2")
                        nc.tensor.matmul(p2[:d_out, :], Wt_s[:d_in, l * 128:l * 128 + d_out],
                                         z[:d_in, :], start=True, stop=True,
                                         skip_group_check=True)
                        cs4 = slice(g4 * 128, (g4 + 4) * 128)
                        nc.scalar.activation(hT_out[:d_out, cs4], p2[:d_out, :], AF.Relu,
                                             bias=bias_s[:d_out, l:l + 1])
                        if l == 1:
                            nc.vector.tensor_add(hT_out[:78, cs4], hT_out[:78, cs4],
                                                 hT_res[:78, cs4])
                        if l == 3:
                            nc.vector.tensor_add(hT_out[:88, cs4], hT_out[:88, cs4],
                                                 hT_res[:88, cs4])
                        for gi in range(4):
                            g = g4 + gi
                            p3 = p3p.tile([128, 128], f32, name=f"p3_{l}_{g}", tag="p3")
                            nc.tensor.transpose(p3[:, :d_out], hT_out[:d_out, g * 128:(g + 1) * 128],
                                                ident_s[:d_out, :d_out])
                            if l < 5:
                                nc.scalar.copy(nm_nxt[:, g * 128:g * 128 + d_out], p3[:, :d_out])
                            else:
                                nc.scalar.copy(ctxnm[:, g * 128:(g + 1) * 128], p3[:, :128])
                        if l < 5:
                            nc.sync.dma_start(
                                hb[g4 * 128:(g4 + 4) * 128, :].rearrange(
                                    "(g p) c -> p g c", p=128),
                                nm_nxt[:, g4 * 128:(g4 + 4) * 128].rearrange(
                                    "p (g c) -> p g c", c=128))
                    if l < 5 and int(os.environ.get("KERNEL_AG", "1")) and not NOCC:
                        nc.gpsimd.collective_compute(
                            "AllGather", mybir.AluOpType.bypass,
                            replica_groups=[list(range(NCORES))],
                            ins=[hb.opt()], outs=[hfp[l].opt()])
                if DBG == 1:
                    lastL = int(os.environ.get("KERNEL_NLAYERS", "6")) - 1
                    hT_last = hT_B if lastL % 2 == 0 else hT_A
                    nc.sync.dma_start(dbg_d.ap(), hT_last[:])
                elif DBG == 2:
                    lastL = int(os.environ.get("KERNEL_NLAYERS", "6")) - 1
                    nm_last = nm1 if lastL % 2 == 0 else nm0
                    nc.sync.dma_start(dbg_d.ap(), nm_last[:])

            hT6 = hT_A  # layer 5 output

            HEAD = bool(int(os.environ.get("KERNEL_HEAD", "1")))
            if not HEAD:
                with tc.tile_pool(name="dummy", bufs=1) as dp:
                    yDm = dp.tile([1, GPC], f32, name="yDm")
                    nc.vector.memset(yDm[:1, :], 0.0)
                    nc.sync.dma_start(y_d.ap().opt(), yDm[:1, :])


            # ---------------- head: passes A+B ----------------
            if HEAD:
             with tc.tile_pool(name="hcA", bufs=1) as hc, \
                 tc.tile_pool(name="hatt", bufs=1) as hatt:
                Wud_s = hc.tile([128, 256], bf, name="Wud_s")
                sumsel_s = hc.tile([128, 16 * GPC], bf, name="sumsel_s")
                bsel_s = hc.tile([16, 128 * GPC], f32, name="bsel_s")
                convw_s = hc.tile([128, 2048], bf, name="convw_s")
                convb_s = hc.tile([32, 1], f32, name="convb_s")
                qemb_s = hc.tile([32, 128], bf, name="qemb_s")
                for name, dst_t in [("qemb", qemb_s), ("W_updown", Wud_s),
                                    ("sumsel", sumsel_s), ("convw", convw_s)]:
                    load_w(dst_t, name, blob=bfull, lay=blay)
                load_w(bsel_s, "bsel")
                load_w(convb_s, "convb")
                # bf16 copy of transposed ctx for the scores matmuls
                nc.vector.tensor_copy(ctxT_bf[:], hT6[:])

                # tanh(attention) for all graphs: [seq%128, seq-tile, graph, l]
                attF = hatt.tile([128, 8, GPC, 128], bf, name="attF")

                # expT/qTb live only through passes A+B1; freed before the conv
                with tc.tile_pool(name="hbig", bufs=1) as hbig:
                 expT = hbig.tile([128, 32 * 1024], bf, name="expT")
                 qTb = hbig.tile([128, GPC * 1024], bf, name="qTb")
                 recipS = hbig.tile([16, 1024], f32, name="recipS")
                 recip_r = hbig.tile([16, 1024], f32r, name="recip_r")

                 # pass A
                 with tc.tile_pool(name="hwA", bufs=2) as hw, \
                     tc.tile_pool(name="pqA", bufs=1, space="PSUM") as pqA, \
                     tc.tile_pool(name="psc", bufs=2, space="PSUM") as psc, \
                     tc.tile_pool(name="psum_sums", bufs=1, space="PSUM") as pss:
                    sums_ps = pss.tile([16, 1024], f32, name="sums_ps")
                    nc.vector.memset(qTb[:], 0.0)
                    for b in range(GPC):
                        # one-hot of this graph's targets: toht[p, j] = (tgt[b, j] == p);
                        # the DMA replicates the DRAM row across 32 partitions
                        tgtb = hw.tile([32, SEQ], bf, name=f"tgtb_{b}", tag="tgtb")
                        nc.sync.dma_start(
                            tgtb[:], dflat[doff["tgt"] + b * SEQ:doff["tgt"] + (b + 1) * SEQ]
                            .rearrange("(x c) -> x c", x=1).broadcast_to([32, SEQ]))
                        toht = hw.tile([32, SEQ], bf, name=f"toh_{b}", tag="toh")
                        nc.vector.tensor_scalar(
                            toht[:], tgtb[:], iota_p[:32, 0:1], None, op0=ALU.is_equal)
                        qp = pqA.tile([128, 1024], f32, name=f"qp_{b}", tag="qp")
                        for cs, cw in ((0, 512), (512, 488)):
                            nc.tensor.matmul(qp[:, cs:cs + cw],
                                             qemb_s[:26, :],
                                             toht[:26, cs:cs + cw],
                                             start=True, stop=True, skip_group_check=True)
                        qbsl = qTb[:, b * 1024:(b + 1) * 1024]
                        nc.scalar.copy(qbsl[:, :1000], qp[:, :1000])
                        for h in range(2):
                            sp = psc.tile([128, 1024], f32, name=f"sp_{b}_{h}", tag="sp")
                            for cs, cw in ((0, 512), (512, 488)):
                                nc.tensor.matmul(
                                    sp[:, cs:cs + cw],
                                    ctxT_bf[:, 256 * b + 128 * h:256 * b + 128 * h + 128],
                                    qbsl[:, cs:cs + cw],
                                    start=True, stop=True, skip_group_check=True)
                            esl = expT[:, (2 * b + h) * 1024:(2 * b + h) * 1024 + 1000]
                            nc.scalar.activation(esl, sp[:, :1000], AF.Exp)
                            for cs, cw in ((0, 512), (512, 488)):
                                nc.tensor.matmul(
                                    sums_ps[:16, cs:cs + cw],
                                    sumsel_s[:, 16 * b:16 * b + 16],
                                    expT[:, (2 * b + h) * 1024 + cs:(2 * b + h) * 1024 + cs + cw],
                                    start=(b == 0 and h == 0), stop=(b == GPC - 1 and h == 1),
                                    skip_group_check=True)
                    nc.vector.reciprocal(recipS[:16, :1000], sums_ps[:16, :1000])
                    nc.gpsimd.dma_start(recip_r[:16, :1000], recipS[:16, :1000])

                 # pass B1: attention mix + tanh, staged into attF
                 with tc.tile_pool(name="hwB", bufs=2) as hw, \
                     tc.tile_pool(name="prb", bufs=1, space="PSUM") as prb, \
                     tc.tile_pool(name="pmix", bufs=1, space="PSUM") as pmix, \
                     tc.tile_pool(name="patt", bufs=2, space="PSUM") as patt:
                        for b in range(GPC):
                            rb = prb.tile([128, 1024], f32, name=f"rb_{b}", tag="rb")
                            for cs, cw in ((0, 512), (512, 488)):
                                nc.tensor.matmul(rb[:, cs:cs + cw],
                                                 bsel_s[:, 128 * b:128 * (b + 1)].bitcast(f32r),
                                                 recip_r[:16, cs:cs + cw],
                                                 start=True, stop=True, skip_group_check=True)
                            mixp = pmix.tile([128, 1024], f32, name=f"mixp_{b}", tag="mixp")
                            for h in range(2):
                                for cs, cw in ((0, 512), (512, 488)):
                                    nc.tensor.matmul(
                                        mixp[:, cs:cs + cw],
                                        ctxnm[:, (2 * b + h) * 128:(2 * b + h + 1) * 128],
                                        expT[:, (2 * b + h) * 1024 + cs:(2 * b + h) * 1024 + cs + cw],
                                        start=(h == 0), stop=(h == 1), skip_group_check=True)
                            rbS = hw.tile([128, 1024], f32, name=f"rbS_{b}", tag="rbS")
                            nc.scalar.copy(rbS[:, :1000], rb[:, :1000])
                            catU = hw.tile([128, 1024], bf, name=f"catU_{b}", tag="catU")
                            nc.vector.tensor_mul(catU[:, :1000], mixp[:, :1000], rbS[:, :1000])
                            nc.vector.memset(catU[:, 1000:1024], 0.0)
                            for qt in range(8):
                                ap_ = patt.tile([128, 128], f32, name=f"attp_{b}_{qt}", tag="attp")
                                nc.tensor.matmul(ap_[:, :], catU[:, 128 * qt:128 * (qt + 1)],
                                                 Wud_s[:, 0:128], start=True, stop=False,
                                                 skip_group_check=True)
                                nc.tensor.matmul(ap_[:, :],
                                                 qTb[:, b * 1024 + 128 * qt:b * 1024 + 128 * (qt + 1)],
                                                 Wud_s[:, 128:256], start=False, stop=True,
                                                 skip_group_check=True)
                                nc.scalar.activation(attF[:, qt, b, :], ap_[:, :], AF.Tanh)
                        if DBG == 3:
                            nc.sync.dma_start(dbg_d.ap()[0:16, 1024:2024], recipS[:16, :1000])
                            nc.gpsimd.dma_start(dbg_d.ap()[:, 2048:3048], expT[:, 0:1000])
                            nc.gpsimd.dma_start(dbg_d.ap()[:, 3048:4048], qTb[:, 0:1000])

                # hbig (expT/qTb) is freed here; pass B2: conv batched over
                # 4 graphs per PSUM accumulation chain
                with tc.tile_pool(name="hwB2", bufs=2) as hwc, \
                     tc.tile_pool(name="pconv", bufs=2, space="PSUM") as pconv, \
                     tc.tile_pool(name="pct", bufs=2, space="PSUM") as pct:
                    for b4 in range(0, GPC, 4):
                        cps = pconv.tile([32, 4 * 121], f32, name=f"cps_{b4}", tag="cps")
                        cps_v = cps[:32, :].rearrange("p (g l) -> p g l", l=121)
                        for k in range(8):
                            for it in range(8):
                                nc.tensor.matmul(
                                    cps_v,
                                    convw_s[:, (k * 8 + it) * 32:(k * 8 + it) * 32 + 32],
                                    attF[:, it, b4:b4 + 4, k:k + 121],
                                    start=(k == 0 and it == 0), stop=(k == 7 and it == 7),
                                    skip_group_check=True)
                        convS = hwc.tile([32, 4 * 121], f32, name=f"convS_{b4}", tag="convS")
                        nc.scalar.activation(convS[:32, :], cps[:32, :], AF.Identity,
                                             bias=convb_s[:32, 0:1])
                        for gi in range(4):
                            b = b4 + gi
                            ct = pct.tile([128, 32], f32, name=f"ct_{b}", tag="ct")
                            nc.tensor.transpose(ct[:121, :32],
                                                convS[:32, gi * 121:(gi + 1) * 121],
                                                ident_s[:32, :32])
                            nc.scalar.copy(convT[:121, 32 * b:32 * (b + 1)], ct[:121, :32])

            # ---------------- head: pooling + final MLPs ----------------
            if HEAD:
             with tc.tile_pool(name="hcB", bufs=1) as hc2, \
                 tc.tile_pool(name="hwC", bufs=1) as hw2, \
                 tc.tile_pool(name="pf", bufs=1, space="PSUM") as pf:
                fc1xtw_s = hc2.tile([128, 4096], bf, name="fc1xtw_s")
                fc1xtb_s = hc2.tile([128, 1], f32, name="fc1xtb_s")
                fcg1w_s = hc2.tile([128, 1024], bf, name="fcg1w_s")
                fcg1b_s = hc2.tile([128, 8], f32, name="fcg1b_s")
                fcg2w_s = hc2.tile([128, 1024], bf, name="fcg2w_s")
                fcg2b_s = hc2.tile([128, 1], f32, name="fcg2b_s")
                fc1w_s = hc2.tile([128, 2048], bf, name="fc1w_s")
                fc1b_s = hc2.tile([128, 8], f32, name="fc1b_s")
                fc2w_s = hc2.tile([128, 4096], bf, name="fc2w_s")
                fc2b_s = hc2.tile([128, 4], f32, name="fc2b_s")
                outw_s = hc2.tile([128, 4], f32, name="outw_s")
                for name, dst_t in [("fc1xtb", fc1xtb_s), ("fcg1b", fcg1b_s),
                                    ("fcg2b", fcg2b_s), ("fc1b", fc1b_s),
                                    ("fc2b", fc2b_s), ("outw", outw_s)]:
                    load_w(dst_t, name)
                for name, dst_t in [("fc1xtw", fc1xtw_s), ("fcg1w", fcg1w_s),
                                    ("fcg2w", fcg2w_s), ("fc1w", fc1w_s),
                                    ("fc2w", fc2w_s)]:
                    load_w(dst_t, name, blob=bfull, lay=blay)

                # g pooling + graph MLP
                hT6_v = hT6[:].rearrange("p (b n) -> p b n", n=256)
                nc.vector.tensor_reduce(gT[:, :GPC], hT6_v, mybir.AxisListType.X,
                                        mybir.AluOpType.max)
                g1_ps = pf.tile([128, 128], f32, name="g1_ps")
                for j in range(8):
                    nc.tensor.matmul(g1_ps[:, 16 * j:16 * (j + 1)],
                                     fcg1w_s[:, 128 * j:128 * (j + 1)], gT[:, :GPC],
                                     start=True, stop=True, skip_group_check=True)
                g1T = hw2.tile([128, 128], bf, name="g1T")
                for j in range(8):
                    nc.scalar.activation(g1T[:, 16 * j:16 * (j + 1)],
                                         g1_ps[:, 16 * j:16 * (j + 1)], AF.Relu,
                                         bias=fcg1b_s[:, j:j + 1])
                g2_ps = pf.tile([128, GPC], f32, name="g2_ps")
                for j in range(8):
                    nc.tensor.matmul(g2_ps[:, :], fcg2w_s[:, 128 * j:128 * (j + 1)],
                                     g1T[:, 16 * j:16 * (j + 1)],
                                     start=(j == 0), stop=(j == 7), skip_group_check=True)
                nc.scalar.activation(g2T[:, :], g2_ps[:, :], AF.Identity,
                                     bias=fcg2b_s[:, 0:1])

                # fc1xt
                convT_v = convT[:121, :].rearrange("p (b o) -> p b o", o=32)
                xt_ps = pf.tile([128, GPC], f32, name="xt_ps")
                for oc in range(32):
                    nc.tensor.matmul(xt_ps[:, :], fc1xtw_s[:121, 128 * oc:128 * (oc + 1)],
                                     convT_v[:, :, oc], start=(oc == 0), stop=(oc == 31),
                                     skip_group_check=True)
                nc.scalar.activation(xtT[:, :], xt_ps[:, :], AF.Identity,
                                     bias=fc1xtb_s[:, 0:1])

                h1_ps = pf.tile([128, 128], f32, name="h1_ps")
                for j in range(8):
                    nc.tensor.matmul(h1_ps[:, 16 * j:16 * (j + 1)],
                                     fc1w_s[:, 128 * j:128 * (j + 1)], g2T[:, :],
                                     start=True, stop=False, skip_group_check=True)
                    nc.tensor.matmul(h1_ps[:, 16 * j:16 * (j + 1)],
                                     fc1w_s[:, 1024 + 128 * j:1024 + 128 * (j + 1)],
                                     xtT[:, :], start=False, stop=True,
                                     skip_group_check=True)
                h1T = hw2.tile([128, 128], bf, name="h1T")
                for j in range(8):
                    nc.scalar.activation(h1T[:, 16 * j:16 * (j + 1)],
                                         h1_ps[:, 16 * j:16 * (j + 1)], AF.Relu,
                                         bias=fc1b_s[:, j:j + 1])
                h2_ps = pf.tile([128, 64], f32, name="h2_ps")
                for j in range(4):
                    for k in range(8):
                        nc.tensor.matmul(
                            h2_ps[:, 16 * j:16 * (j + 1)],
                            fc2w_s[:, 128 * (4 * k + j):128 * (4 * k + j + 1)],
                            h1T[:, 16 * k:16 * (k + 1)],
                            start=(k == 0), stop=(k == 7), skip_group_check=True)
                h2T = hw2.tile([128, 64], f32, name="h2T")
                for j in range(4):
                    nc.scalar.activation(h2T[:, 16 * j:16 * (j + 1)],
                                         h2_ps[:, 16 * j:16 * (j + 1)], AF.Relu,
                                         bias=fc2b_s[:, j:j + 1])
                y_ps = pf.tile([1, GPC], f32, name="y_ps")
                for k in range(4):
                    nc.tensor.matmul(y_ps[:1, :], outw_s[:, k:k + 1],
                                     h2T[:, 16 * k:16 * (k + 1)],
                                     start=(k == 0), stop=(k == 3), skip_group_check=True)
                yS = hw2.tile([1, GPC], f32, name="yS")
                nc.scalar.activation(yS[:1, :], y_ps[:1, :], AF.Identity,
                                     bias=float(outb))
                nc.sync.dma_start(y_d.ap().opt(), yS[:1, :])
                if DBG == 3:
                    nc.sync.dma_start(dbg_d.ap()[:, 0:512], convT[:])
                    nc.sync.dma_start(dbg_d.ap()[:, 512:528], gT[:])
                    nc.sync.dma_start(dbg_d.ap()[:, 528:544], g2T[:])
                    nc.sync.dma_start(dbg_d.ap()[:, 544:560], xtT[:])

    nc.compile()
    return nc


def kernel(**inputs) -> np.ndarray:
    from concourse.bass_utils import run_bass_kernel_spmd
    import jax
    try:
        # cache the XLA executable on disk: run_bass_via_pjrt re-jits a fresh
        # closure per call, and without this every call repays ~0.9s of
        # BIR verify + NEFF wrap even with a warm neuronxcc cache
        jax.config.update("jax_compilation_cache_dir", "/tmp/jaxcache")
        jax.config.update("jax_persistent_cache_min_compile_time_secs", 0.0)
        jax.config.update("jax_persistent_cache_min_entry_size_bytes", -1)
    except Exception as e:
        print(f"jax compilation cache unavailable: {e}", flush=True)

    t0 = time.time()
    struct, per_core, outb = _host_prep(inputs)
    t1 = time.time()
    nc = _build(struct, outb)
    t2 = time.time()
    print(f"host_prep {t1 - t0:.3f}s  build+compile {t2 - t1:.3f}s", flush=True)

    in_maps = [dict(per_core[c]) for c in range(NCORES)]

    trace = bool(int(os.environ.get("KERNEL_TRACE", "0")))
    t_run = time.time()
    try:
        res = run_bass_kernel_spmd(nc, in_maps, core_ids=list(range(NCORES)), trace=False)
    except Exception as e:  # transient NRT device errors observed under axon; retry once
        print(f"spmd attempt 1 failed ({type(e).__name__}); retrying once", flush=True)
        time.sleep(5)
        res = run_bass_kernel_spmd(nc, in_maps, core_ids=list(range(NCORES)), trace=False)
    print(f"spmd wall #1 (compile+transfer+exec): {time.time() - t_run:.3f}s", flush=True)
    if trace:
        # second run reuses the jit/NEFF cache: wall is transfer+exec only
        best = None
        for _ in range(int(os.environ.get("KERNEL_REPS", "10"))):
            t_run = time.time()
            res = run_bass_kernel_spmd(nc, in_maps, core_ids=list(range(NCORES)), trace=False)
            wall2 = time.time() - t_run
            print(f"spmd wall (transfer+exec): {wall2:.3f}s", flush=True)
            best = wall2 if best is None else min(best, wall2)
        print(f"HW exec time: {int(best * 1e9)} ns (wall-clock upper bound; "
              f"NTFF profiling unavailable under this axon tunnel)")
    if int(os.environ.get("KERNEL_DEBUG", "0")):
        np.save("/tmp/dbg.npy", np.stack([res.results[c]["dbg"] for c in range(NCORES)]))
    out = np.concatenate([res.results[c]["y"] for c in range(NCORES)], axis=0)
    return out.astype(np.float32)
